# revision 12
# baseline (speedup 1.0000x reference)
"""Trainium2 Bass kernel for nn_Crossings (segment-pair intersection counts per graph).

Strategy (8 NeuronCores, SPMD). TRN2 has no usable bulk per-element random
gather (indirect DMA is descriptor-rate-bound at ~2.3G desc/s and its
multi-offset form miscompiles; GPSIMD gathers are int16/per-16-partition MoE
primitives), so the node-position gather is done as host-side input
marshalling and the device runs a pure streaming kernel:

  - Host: sort the 16M pairs by graph id (counting sort via
    argsort(batch_index[s1]) — index-only work), translate all four endpoint
    coordinates by -p1 (the predicate is shift-invariant, saving two planes
    and four device subtractions; host fp32 subtraction matches the
    reference's own first-level fp32 arithmetic), emit six fp16 coordinate
    planes (S,T = p2-p1; A,B = p3-p1; C,D = p4-p1) padded so every
    partition-row of slots belongs to exactly one graph, sharded evenly
    across the 8 cores.
  - Device (per core): stream the 6 planes tile-by-tile ([128, 4096] fp16,
    DVE tensor_tensor runs in 2x mode), evaluate the proper-intersection
    orientation test with 16 TT ops + 1 tensor_scalar compare, reduce each
    partition-row to a crossing count -> rowsums [128, n_tiles].
  - Host: map padded rows back to graphs, accumulate in float64,
    return float32 [128].

Accuracy: fp16 coordinate rounding flips ~0.1% of near-degenerate
orientation tests; measured relative error ~7e-4 on the [128] output
(fp32 planes give ~3e-6 at ~5x the runtime; set DTYPE = "f32").
"""
import sys

sys.path.insert(0, "/opt/trn_rl_repo")

import numpy as np

import concourse.bacc as bacc
import concourse.mybir as mybir
import concourse.tile as tile
from concourse import bass
from concourse.bass_utils import run_bass_kernel_spmd

EPS = 1e-5
NUM_GRAPHS = 128
N_CORES = 8
P = 128          # SBUF partitions
F = 4096         # free-dim tile width (slots per partition-row per tile)
ROW = F          # slots per partition-row
TILE_SLOTS = P * F

DTYPE = "f16"   # "f16" | "bf16" | "f32"


def _np_dtype():
    if DTYPE == "bf16":
        import ml_dtypes
        return ml_dtypes.bfloat16
    if DTYPE == "f16":
        return np.float16
    return np.float32


def _mybir_dtype():
    import concourse.mybir as _mb
    return {"bf16": _mb.dt.bfloat16, "f16": _mb.dt.float16, "f32": _mb.dt.float32}[DTYPE]


def _build_program(n_tiles: int, repeats: int = 1):
    nc = bacc.Bacc()
    dt = _mybir_dtype()
    f32 = mybir.dt.float32
    op = mybir.AluOpType

    streams = nc.declare_dram_parameter(
        "streams", [6, n_tiles, P, F], dt, isOutput=False
    )
    rowsums = nc.declare_dram_parameter(
        "rowsums", [P, n_tiles], f32, isOutput=True
    )

    with tile.TileContext(nc) as tc:
        with (
            tc.tile_pool(name="io", bufs=3) as iop,
            tc.tile_pool(name="tmp", bufs=1) as tmp,
            tc.tile_pool(name="accp", bufs=1) as accp,
        ):
            acc = accp.tile([P, n_tiles], f32)
            for t in [tt for _ in range(repeats) for tt in range(n_tiles)]:
                pl = []
                for s in range(6):
                    st = iop.tile([P, F], dt, tag=f"in{s}")
                    nc.sync.dma_start(out=st[:], in_=streams[s, t])
                    pl.append(st)
                # host pre-translated by -p1:  S=p2-p1, (A,B)=p3-p1, (C,D)=p4-p1
                S, T, A, B, C, D = pl

                def tt_(tag, a, b, o):
                    r = tmp.tile([P, F], dt, tag=tag)
                    nc.vector.tensor_tensor(out=r[:], in0=a[:], in1=b[:], op=o)
                    return r

                u = tt_("u", C, A, op.subtract)     # x4 - x3
                v = tt_("v", D, B, op.subtract)     # y4 - y3

                # d1 = cross(p4-p3, p1-p3) = v*A - u*B
                m1 = tt_("p1", v, A, op.mult)
                m2 = tt_("p2", u, B, op.mult)
                d1 = tt_("d1", m1, m2, op.subtract)
                # k = cross(p4-p3, p2-p1) = u*T - v*S
                k1 = tt_("p1", u, T, op.mult)
                k2 = tt_("p2", v, S, op.mult)
                kk = tt_("kk", k1, k2, op.subtract)
                # d3 = cross(p2-p1, p3-p1) = S*B - T*A
                m5 = tt_("p1", S, B, op.mult)
                m6 = tt_("p2", T, A, op.mult)
                d3 = tt_("d3", m5, m6, op.subtract)

                d2 = tt_("p1", d1, kk, op.add)      # d2 = d1 + k
                d4 = tt_("p2", d3, kk, op.subtract)  # d4 = d3 - k
                t1 = tt_("u", d1, d2, op.mult)
                t2 = tt_("v", d3, d4, op.mult)
                mx = tt_("d1", t1, t2, op.max)

                xing = tmp.tile([P, F], dt, tag="kk")
                nc.vector.tensor_scalar(
                    out=xing[:], in0=mx[:], scalar1=-EPS, scalar2=None, op0=op.is_lt
                )
                nc.vector.tensor_reduce(
                    out=acc[:, t : t + 1],
                    in_=xing[:],
                    op=op.add,
                    axis=mybir.AxisListType.X,
                )
            nc.sync.dma_start(out=rowsums[:], in_=acc[:])
    nc.finalize()
    return nc


def _prepare(node_pos, batch_index, edge_pair_index):
    """Host marshalling. Returns (in_maps, row2graph [N_CORES, P, n_tiles], n_tiles)."""
    npos = np.asarray(node_pos, dtype=np.float32)
    bidx = np.asarray(batch_index)
    epi = np.asarray(edge_pair_index)

    # reference: (s1, s2), (e1, e2) = edge_pair_index
    s1 = epi[0, 0].astype(np.int64)
    s2 = epi[0, 1].astype(np.int64)
    e1 = epi[1, 0].astype(np.int64)
    e2 = epi[1, 1].astype(np.int64)

    g = bidx[s1].astype(np.int32)         # graph id per pair
    order = np.argsort(g, kind="stable")  # counting-style sort by graph
    s1, e1, s2, e2 = s1[order], e1[order], s2[order], e2[order]
    g_sorted = g[order]

    counts = np.bincount(g_sorted, minlength=NUM_GRAPHS)
    # pad each graph's range to a multiple of ROW so every partition-row
    # belongs to exactly one graph
    padded = ((counts + ROW - 1) // ROW) * ROW
    total = int(padded.sum())
    n_rows_total = total // ROW
    rows_per_core = int(np.ceil(n_rows_total / N_CORES))
    n_tiles = int(np.ceil(rows_per_core / P))
    core_slots = n_tiles * TILE_SLOTS

    row_graph = np.repeat(np.arange(NUM_GRAPHS), padded // ROW)  # graph per row

    starts = np.zeros(NUM_GRAPHS + 1, np.int64)
    starts[1:] = np.cumsum(padded)
    src_starts = np.zeros(NUM_GRAPHS + 1, np.int64)
    src_starts[1:] = np.cumsum(counts)
    pos = np.empty(len(s1), np.int64)
    for gg in range(NUM_GRAPHS):
        a, b = src_starts[gg], src_starts[gg + 1]
        pos[a:b] = np.arange(a, b) - a + starts[gg]

    ndt = _np_dtype()
    planes = np.zeros((6, N_CORES * core_slots), ndt)
    x1, y1 = npos[s1, 0], npos[s1, 1]
    coords = (
        npos[e1, 0] - x1, npos[e1, 1] - y1,   # S, T  (p2 - p1)
        npos[s2, 0] - x1, npos[s2, 1] - y1,   # A, B  (p3 - p1)
        npos[e2, 0] - x1, npos[e2, 1] - y1,   # C, D  (p4 - p1)
    )
    for i in range(6):
        planes[i, pos] = coords[i].astype(ndt)

    per_core = planes.reshape(6, N_CORES, n_tiles, P, F).transpose(1, 0, 2, 3, 4)
    in_maps = [{"streams": np.ascontiguousarray(per_core[c])} for c in range(N_CORES)]

    # device row sums land at rowsums[p, t]; global row id = c*(n_tiles*P) + t*P + p
    rid = (
        np.arange(N_CORES)[:, None, None] * (n_tiles * P)
        + np.arange(n_tiles)[None, None, :] * P
        + np.arange(P)[None, :, None]
    )
    row2graph = np.where(rid < n_rows_total, row_graph[np.minimum(rid, n_rows_total - 1)], -1)
    return in_maps, row2graph, n_tiles


def kernel(node_pos, edge_index, apsp, batch_index, edge_pair_index):
    in_maps, row2graph, n_tiles = _prepare(node_pos, batch_index, edge_pair_index)
    nc = _build_program(n_tiles)
    res = run_bass_kernel_spmd(nc, in_maps, list(range(N_CORES))).results

    out = np.zeros(NUM_GRAPHS, np.float64)
    for c in range(N_CORES):
        rs = res[c]["rowsums"].astype(np.float64)  # [P, n_tiles]
        valid = row2graph[c] >= 0
        np.add.at(out, row2graph[c][valid], rs[valid])
    return out.astype(np.float32)


# revision 14
# speedup vs baseline: 1.3135x; 1.3135x over previous
"""Trainium2 Bass kernel for nn_Crossings (segment-pair intersection counts per graph).

Strategy (8 NeuronCores, SPMD). TRN2 has no usable bulk per-element random
gather (indirect DMA is descriptor-rate-bound at ~2.3G desc/s and its
multi-offset form miscompiles; GPSIMD gathers are int16/per-16-partition MoE
primitives), so the node-position gather is done as host-side input
marshalling and the device runs a pure streaming kernel:

  - Host: sort the 16M pairs by graph id (counting sort via
    argsort(batch_index[s1]) — index-only work), translate all four endpoint
    coordinates by -p1 (the predicate is shift-invariant, saving two planes
    and four device subtractions; host fp32 subtraction matches the
    reference's own first-level fp32 arithmetic), emit six fp16 coordinate
    planes (S,T = p2-p1; A,B = p3-p1; C,D = p4-p1) padded so every
    partition-row of slots belongs to exactly one graph, sharded evenly
    across the 8 cores.
  - Device (per core): stream the 6 planes tile-by-tile ([128, 4096] fp16,
    DVE tensor_tensor runs in 2x mode), evaluate the proper-intersection
    orientation test with 16 TT ops + 1 tensor_scalar compare, reduce each
    partition-row to a crossing count -> rowsums [128, n_tiles].
  - Host: map padded rows back to graphs, accumulate in float64,
    return float32 [128].

Accuracy: fp16 coordinate rounding flips ~0.1% of near-degenerate
orientation tests; measured relative error ~7e-4 on the [128] output
(fp32 planes give ~3e-6 at ~5x the runtime; set DTYPE = "f32").
"""
import sys

sys.path.insert(0, "/opt/trn_rl_repo")

import numpy as np

import concourse.bacc as bacc
import concourse.mybir as mybir
import concourse.tile as tile
from concourse import bass
from concourse.bass_utils import run_bass_kernel_spmd

EPS = 1e-5
NUM_GRAPHS = 128
N_CORES = 8
P = 128          # SBUF partitions
F = 4096         # free-dim tile width (slots per partition-row per tile)
ROW = F          # slots per partition-row
TILE_SLOTS = P * F

DTYPE = "f16"   # "f16" | "bf16" | "f32"


def _np_dtype():
    if DTYPE == "bf16":
        import ml_dtypes
        return ml_dtypes.bfloat16
    if DTYPE == "f16":
        return np.float16
    return np.float32


def _mybir_dtype():
    import concourse.mybir as _mb
    return {"bf16": _mb.dt.bfloat16, "f16": _mb.dt.float16, "f32": _mb.dt.float32}[DTYPE]


def _build_program(n_tiles: int, repeats: int = 1):
    nc = bacc.Bacc()
    dt = _mybir_dtype()
    f32 = mybir.dt.float32
    op = mybir.AluOpType

    streams = nc.declare_dram_parameter(
        "streams", [6, n_tiles, P, F], dt, isOutput=False
    )
    rowsums = nc.declare_dram_parameter(
        "rowsums", [P, n_tiles], f32, isOutput=True
    )

    with tile.TileContext(nc) as tc:
        with (
            tc.tile_pool(name="io", bufs=3) as iop,
            tc.tile_pool(name="tmp", bufs=1) as tmp,
            tc.tile_pool(name="accp", bufs=1) as accp,
        ):
            acc = accp.tile([P, n_tiles], f32)
            eps_col = accp.tile([P, 1], f32, tag="eps")
            nc.vector.memset(eps_col[:], EPS)
            for t in [tt for _ in range(repeats) for tt in range(n_tiles)]:
                pl = []
                for s in range(6):
                    st = iop.tile([P, F], dt, tag=f"in{s}")
                    nc.sync.dma_start(out=st[:], in_=streams[s, t])
                    pl.append(st)
                # host pre-translated by -p1:  S=p2-p1, (A,B)=p3-p1, (C,D)=p4-p1
                S, T, A, B, C, D = pl

                def tt_(tag, a, b, o):
                    r = tmp.tile([P, F], dt, tag=tag)
                    nc.vector.tensor_tensor(out=r[:], in0=a[:], in1=b[:], op=o)
                    return r

                # In p1-translated coordinates (p1 = origin) the four
                # orientation values reduce to three pairwise cross products:
                #   c34 = cross(p3,p4) = d1,  c32 = cross(p3,p2) = -d3,
                #   c42 = cross(p4,p2) = -d4,  d2 = c42 + c34 - c32,
                #   t1 = d1*d2 = c34*d2,  t2 = d3*d4 = c32*c42.
                m1 = tt_("p1", A, D, op.mult)
                m2 = tt_("p2", B, C, op.mult)
                c34 = tt_("c34", m1, m2, op.subtract)
                m3 = tt_("p1", A, T, op.mult)
                m4 = tt_("p2", B, S, op.mult)
                c32 = tt_("c32", m3, m4, op.subtract)
                m5 = tt_("p1", C, T, op.mult)
                m6 = tt_("p2", D, S, op.mult)
                c42 = tt_("c42", m5, m6, op.subtract)

                e = tt_("p1", c42, c34, op.add)
                d2 = tt_("p2", e, c32, op.subtract)
                t2 = tt_("p1", c32, c42, op.mult)
                t1 = tt_("c32", c34, d2, op.mult)
                mx = tt_("c34", t1, t2, op.max)

                # crossing iff mx < -EPS  <=>  sign(mx + EPS) == -1
                # (-EPS is not fp16-representable, so mx + EPS != 0 always);
                # row count = (F - sum(sign)) / 2, applied on the host.
                sgn = tmp.tile([P, F], dt, tag="c42")
                nc.scalar.activation(
                    out=sgn[:],
                    in_=mx[:],
                    func=mybir.ActivationFunctionType.Sign,
                    bias=eps_col[:],
                    accum_out=acc[:, t : t + 1],
                )
            nc.sync.dma_start(out=rowsums[:], in_=acc[:])
    nc.finalize()
    return nc


def _prepare(node_pos, batch_index, edge_pair_index):
    """Host marshalling. Returns (in_maps, row2graph [N_CORES, P, n_tiles], n_tiles)."""
    npos = np.asarray(node_pos, dtype=np.float32)
    bidx = np.asarray(batch_index)
    epi = np.asarray(edge_pair_index)

    # reference: (s1, s2), (e1, e2) = edge_pair_index
    s1 = epi[0, 0].astype(np.int64)
    s2 = epi[0, 1].astype(np.int64)
    e1 = epi[1, 0].astype(np.int64)
    e2 = epi[1, 1].astype(np.int64)

    g = bidx[s1].astype(np.int32)         # graph id per pair
    order = np.argsort(g, kind="stable")  # counting-style sort by graph
    s1, e1, s2, e2 = s1[order], e1[order], s2[order], e2[order]
    g_sorted = g[order]

    counts = np.bincount(g_sorted, minlength=NUM_GRAPHS)
    # pad each graph's range to a multiple of ROW so every partition-row
    # belongs to exactly one graph
    padded = ((counts + ROW - 1) // ROW) * ROW
    total = int(padded.sum())
    n_rows_total = total // ROW
    rows_per_core = int(np.ceil(n_rows_total / N_CORES))
    n_tiles = int(np.ceil(rows_per_core / P))
    core_slots = n_tiles * TILE_SLOTS

    row_graph = np.repeat(np.arange(NUM_GRAPHS), padded // ROW)  # graph per row

    starts = np.zeros(NUM_GRAPHS + 1, np.int64)
    starts[1:] = np.cumsum(padded)
    src_starts = np.zeros(NUM_GRAPHS + 1, np.int64)
    src_starts[1:] = np.cumsum(counts)
    pos = np.empty(len(s1), np.int64)
    for gg in range(NUM_GRAPHS):
        a, b = src_starts[gg], src_starts[gg + 1]
        pos[a:b] = np.arange(a, b) - a + starts[gg]

    ndt = _np_dtype()
    planes = np.zeros((6, N_CORES * core_slots), ndt)
    x1, y1 = npos[s1, 0], npos[s1, 1]
    coords = (
        npos[e1, 0] - x1, npos[e1, 1] - y1,   # S, T  (p2 - p1)
        npos[s2, 0] - x1, npos[s2, 1] - y1,   # A, B  (p3 - p1)
        npos[e2, 0] - x1, npos[e2, 1] - y1,   # C, D  (p4 - p1)
    )
    for i in range(6):
        planes[i, pos] = coords[i].astype(ndt)

    per_core = planes.reshape(6, N_CORES, n_tiles, P, F).transpose(1, 0, 2, 3, 4)
    in_maps = [{"streams": np.ascontiguousarray(per_core[c])} for c in range(N_CORES)]

    # device row sums land at rowsums[p, t]; global row id = c*(n_tiles*P) + t*P + p
    rid = (
        np.arange(N_CORES)[:, None, None] * (n_tiles * P)
        + np.arange(n_tiles)[None, None, :] * P
        + np.arange(P)[None, :, None]
    )
    row2graph = np.where(rid < n_rows_total, row_graph[np.minimum(rid, n_rows_total - 1)], -1)
    return in_maps, row2graph, n_tiles


def kernel(node_pos, edge_index, apsp, batch_index, edge_pair_index):
    in_maps, row2graph, n_tiles = _prepare(node_pos, batch_index, edge_pair_index)
    nc = _build_program(n_tiles)
    res = run_bass_kernel_spmd(nc, in_maps, list(range(N_CORES))).results

    out = np.zeros(NUM_GRAPHS, np.float64)
    for c in range(N_CORES):
        # device accumulated sum(sign(mx+EPS)); crossing count per row is
        # (F - sum_sign) / 2
        rs = (F - res[c]["rowsums"].astype(np.float64)) / 2.0  # [P, n_tiles]
        valid = row2graph[c] >= 0
        np.add.at(out, row2graph[c][valid], rs[valid])
    return out.astype(np.float32)
